# revision 52
# baseline (speedup 1.0000x reference)
"""Trainium2 Bass kernel for a transformer encoder layer.

B=4, S=2048, D=1024, H=16 heads (HD=64), PF=4096, fp32 I/O.

Sharding: 8 cores, core c handles batch c//2, query seq-half c%2 (1024
tokens). Each core computes K/V over its batch's full 2048-token sequence
(duplicated within the pair; ~12% extra flops) so no collectives are needed.

Precision plan:
- Attention path in fp8e4 with DoubleRow matmuls (2 contraction subtiles
  per instruction, 2x PE throughput): QKV projections, attn@V, out-proj.
  Wq/Wk/Wv/Wo are scaled x32 host-side so their values use the fp8 range;
  the compensating 1/1024 factors are folded into the exp scale (scores)
  and the out-proj eviction. Scores (64-deep contraction) stay bf16 -
  DoubleRow cannot speed a 64-row contraction.
- exp computed as exp(s/8 - 2) so expS fits fp8e4 (max 240); the -2 shift
  cancels in the softmax normalization (ones-column denominator trick).
- FFN and residual/LN stay bf16/fp32 for accuracy.
- src streamed in as bf16; transposes run in bf16 (1 cyc/row vs 2 for f32).
- x^T and src1 stay SBUF-resident (no DRAM round trips).
"""

import numpy as np

D = 1024
S2 = 2048
SQ = 1024
PF = 4096
H = 16
HD = 64
DK = D // 128
PFK = PF // 128
NG = 4                 # head groups
HPG = H // NG
GW = HPG * HD          # 256 dims per group
GM = GW // 128
WS = 32.0              # host-side weight scale for fp8 Wq/Wk/Wv/Wo
SCALE_EXP = 1.0 / (8.0 * WS * WS)   # undo q,k scales and apply 1/sqrt(HD)
EXP_BIAS = -2.0
EPS = 1e-5
N_CORES = 8

_CACHE = {}


def _build():
    import concourse.bass as bass
    import concourse.mybir as mybir
    import concourse.tile as tile
    from concourse import bacc
    from concourse.masks import make_identity

    f32 = mybir.dt.float32
    bf16 = mybir.dt.bfloat16
    f8 = mybir.dt.float8e4
    AF = mybir.ActivationFunctionType
    ALU = mybir.AluOpType
    DR = mybir.MatmulPerfMode.DoubleRow

    nc = bacc.Bacc("TRN2", target_bir_lowering=False, debug=False, num_devices=N_CORES)

    def din(name, shape, dt=f32):
        return nc.dram_tensor(name, shape, dt, kind="ExternalInput")

    src_q = din("src_q", [SQ, D], bf16)
    src_o = din("src_o", [SQ, D], bf16)
    Wq = din("Wq", [D, D], f8)
    Wk = din("Wk", [D, D], f8)
    Wv = din("Wv", [D, D], f8)
    Wo = din("Wo", [D, D], f8)
    W1 = din("W1", [D, PF], bf16)
    W2 = din("W2", [PF, D], bf16)
    bq = din("bq", [D])
    bk = din("bk", [D])
    bv = din("bv", [D])
    bo = din("bo", [D])
    bf1 = din("bf1", [PF])
    bf2 = din("bf2", [D])
    g1 = din("g1", [D])
    b1 = din("b1", [D])
    g2 = din("g2", [D])
    b2 = din("b2", [D])
    out = nc.dram_tensor("out", [SQ, D], f32, kind="ExternalOutput")

    def bc_ap(vec, n):
        return bass.AP(tensor=vec, offset=0, ap=[[0, 128], [1, n]])

    def col_ap(vec, m):
        return bass.AP(tensor=vec, offset=0, ap=[[1, 128], [128, m]])

    with tile.TileContext(nc) as tc:
        import contextlib

        with contextlib.ExitStack() as ctx:
            consts = ctx.enter_context(tc.tile_pool(name="consts", bufs=1))

            identity = consts.tile([128, 128], bf16)
            make_identity(nc, identity)

            bq_col = consts.tile([128, DK], f32)
            nc.sync.dma_start(out=bq_col, in_=col_ap(bq, DK))
            bk_col = consts.tile([128, DK], f32)
            nc.sync.dma_start(out=bk_col, in_=col_ap(bk, DK))
            bf1_col = consts.tile([128, PFK], f32)
            nc.sync.dma_start(out=bf1_col, in_=col_ap(bf1, PFK))

            eps_t = consts.tile([128, 1], f32)
            nc.vector.memset(eps_t, EPS)
            expb_t = consts.tile([128, 1], f32)
            nc.vector.memset(expb_t, EXP_BIAS)

            # src1 / src1^T span out-proj -> FFN; allocated up front
            src1p = ctx.enter_context(tc.tile_pool(name="src1p", bufs=1))
            src1_sb = src1p.tile([128, SQ // 128, D], bf16)  # 2MB
            src1T = src1p.tile([128, DK, SQ], bf16)          # 2MB

            mid = ctx.enter_context(contextlib.ExitStack())
            # activations that span phase0 -> out-proj (freed before FFN):
            # src_q (+bo) residual (bf16), x^T (fp8), Wo (fp8)
            actp = mid.enter_context(tc.tile_pool(name="actp", bufs=1))
            src_q_sb = actp.tile([128, SQ // 128, D], bf16)  # 2MB
            xts = actp.tile([128, DK, SQ], f8)        # 1MB
            wo_f = actp.tile([128, DK, D], f8)
            nc.gpsimd.dma_start(out=wo_f, in_=Wo.rearrange("(a p) n -> p a n", p=128))

            def ln_apply(r_row, acc1, acc2, g_bc, b_bc, out_tile, tmp_pool):
                """LayerNorm given acc1=sum(r) and acc2=sum(r^2) per partition.

                The moment sums come for free from accum_out on the passes
                that produce r, so this needs only two full-width DVE passes.
                The tiny [128,1] moment arithmetic runs on gpsimd (idle).
                """
                mu = tmp_pool.tile([128, 1], f32, tag="ln_mu")
                nc.vector.tensor_scalar_mul(out=mu, in0=acc1, scalar1=1.0 / D)
                musq = tmp_pool.tile([128, 1], f32, tag="ln_musq")
                nc.vector.tensor_mul(out=musq, in0=mu, in1=mu)
                var = tmp_pool.tile([128, 1], f32, tag="ln_var")
                nc.vector.scalar_tensor_tensor(
                    out=var,
                    in0=acc2,
                    scalar=1.0 / D,
                    in1=musq,
                    op0=ALU.mult,
                    op1=ALU.subtract,
                )
                rstd = tmp_pool.tile([128, 1], f32, tag="ln_rstd")
                nc.scalar.activation(
                    out=rstd, in_=var, func=AF.Sqrt, bias=eps_t, scale=1.0
                )
                nc.vector.reciprocal_approx_fast(out=rstd, in_=rstd)
                nc.vector.scalar_tensor_tensor(
                    out=out_tile,
                    in0=r_row,
                    scalar=mu,
                    in1=g_bc,
                    op0=ALU.subtract,
                    op1=ALU.mult,
                )
                nc.vector.scalar_tensor_tensor(
                    out=out_tile,
                    in0=out_tile,
                    scalar=rstd,
                    in1=b_bc,
                    op0=ALU.mult,
                    op1=ALU.add,
                )

            # attention-scope tensors: QKV weights (fp8), src^T (fp8), bo
            attn_outer = mid.enter_context(contextlib.ExitStack())
            wqkvp = attn_outer.enter_context(tc.tile_pool(name="wqkvp", bufs=1))
            wk_f = wqkvp.tile([128, DK, D], f8)
            nc.sync.dma_start(out=wk_f, in_=Wk.rearrange("(a p) n -> p a n", p=128))
            wq_f = wqkvp.tile([128, DK, D], f8)
            nc.sync.dma_start(out=wq_f, in_=Wq.rearrange("(a p) n -> p a n", p=128))
            wv_f = wqkvp.tile([128, DK, D], f8)
            nc.sync.dma_start(out=wv_f, in_=Wv.rearrange("(a p) n -> p a n", p=128))
            srcT = wqkvp.tile([128, DK, S2], f8)      # 2MB
            bo_bc = wqkvp.tile([128, D], f32)
            nc.gpsimd.dma_start(out=bo_bc, in_=bc_ap(bo, D))

            # ============ Phase 0: transpose src (bf16 in, fp8 out) ============
            with tc.tile_pool(name="psA", bufs=1, space="PSUM") as psA:
                # PE warm-up: ramp the clock while the first src DMAs land.
                for w in range(16):
                    wps = psA.tile([128, 512], bf16, tag="tpsb", bufs=4)
                    for j in range(4):
                        nc.tensor.transpose(
                            wps[:, j * 128 : (j + 1) * 128], identity, identity
                        )

                with tc.tile_pool(name="ph0", bufs=2) as ph0:
                    for blk in range(4):
                        sts = []
                        for j in range(4):
                            row0 = (blk % 2) * 512 + j * 128
                            if blk < 2:
                                st = src_q_sb[:, blk * 4 + j, :]
                                nc.sync.dma_start(
                                    out=st, in_=src_q[row0 : row0 + 128, :]
                                )
                            else:
                                st = ph0.tile([128, D], bf16, tag="src_ld", bufs=8)
                                nc.sync.dma_start(
                                    out=st, in_=src_o[row0 : row0 + 128, :]
                                )
                            sts.append(st)
                        base = blk * 512
                        for k in range(DK):
                            ps = psA.tile([128, 512], bf16, tag="tpsb", bufs=4)
                            for j in range(4):
                                nc.tensor.transpose(
                                    ps[:, j * 128 : (j + 1) * 128],
                                    sts[j][:, k * 128 : (k + 1) * 128],
                                    identity,
                                )
                            nc.vector.tensor_copy(
                                out=srcT[:, k, base : base + 512], in_=ps
                            )
                # fold bo into the resident src_q copy (after its transposes)
                for j in range(SQ // 128):
                    nc.vector.tensor_add(
                        out=src_q_sb[:, j, :], in0=src_q_sb[:, j, :], in1=bo_bc
                    )

            # ============ attention ============
            with contextlib.ExitStack() as attn_ctx:
                psum = attn_ctx.enter_context(
                    tc.tile_pool(name="psB", bufs=1, space="PSUM")
                )
                acts = attn_ctx.enter_context(tc.tile_pool(name="acts", bufs=1))
                bv_bc = acts.tile([128, D], f32)
                nc.gpsimd.dma_start(out=bv_bc, in_=bc_ap(bv, D))

                grp = attn_ctx.enter_context(tc.tile_pool(name="grp", bufs=2))
                expp = attn_ctx.enter_context(tc.tile_pool(name="expp", bufs=2))
                nrm = attn_ctx.enter_context(tc.tile_pool(name="nrm", bufs=2))

                for g in range(NG):
                    gc0 = g * GW

                    # -- KT_g [GW, S2] fp8 (values x32) --
                    KT = grp.tile([128, GM, S2], f8, tag="KT")
                    for m in range(GM):
                        mc = gc0 + m * 128
                        for nn in range(S2 // 1024):
                            ps = psum.tile([128, 1024], f32, tag="big", bufs=2)
                            for hf in range(2):
                                c0 = nn * 1024 + hf * 512
                                for kp in range(DK // 2):
                                    nc.tensor.matmul(
                                        ps[:, hf * 512 : (hf + 1) * 512],
                                        wk_f[:, 2 * kp : 2 * kp + 2, mc : mc + 128],
                                        srcT[:, 2 * kp : 2 * kp + 2, c0 : c0 + 512],
                                        start=(kp == 0),
                                        stop=(kp == DK // 2 - 1),
                                        perf_mode=DR,
                                    )
                            nc.vector.tensor_scalar_add(
                                out=KT[:, m, nn * 1024 : (nn + 1) * 1024],
                                in0=ps,
                                scalar1=bk_col[:, mc // 128 : mc // 128 + 1],
                            )

                    # -- QT_g [GW, SQ] fp8 (values x32) --
                    QT = grp.tile([128, GM, SQ], f8, tag="QT")
                    for m in range(GM):
                        mc = gc0 + m * 128
                        ps = psum.tile([128, 1024], f32, tag="big", bufs=2)
                        for hf in range(2):
                            for kp in range(DK // 2):
                                nc.tensor.matmul(
                                    ps[:, hf * 512 : (hf + 1) * 512],
                                    wq_f[:, 2 * kp : 2 * kp + 2, mc : mc + 128],
                                    srcT[:, 2 * kp : 2 * kp + 2, hf * 512 : (hf + 1) * 512],
                                    start=(kp == 0),
                                    stop=(kp == DK // 2 - 1),
                                    perf_mode=DR,
                                )
                        nc.vector.tensor_scalar_add(
                            out=QT[:, m, :],
                            in0=ps,
                            scalar1=bq_col[:, mc // 128 : mc // 128 + 1],
                        )

                    # -- V_g fp8 [S2, HPG, 80], each head [v|1|pad] --
                    # (stride 80 keeps the DoubleRow pair step 16B-aligned)
                    VP = 80
                    V = grp.tile([128, S2 // 128, HPG, VP], f8, tag="V")
                    nc.vector.memset(V[:, :, :, HD : HD + 1], 1.0)
                    bv_v = bv_bc.rearrange("p (h d) -> p h d", h=H)
                    for ms in range(S2 // 128):
                        ps = psum.tile([128, 1024], f32, tag="big", bufs=2)
                        for kp in range(DK // 2):
                            nc.tensor.matmul(
                                ps[:, 0:GW],
                                srcT[:, 2 * kp : 2 * kp + 2, ms * 128 : (ms + 1) * 128],
                                wv_f[:, 2 * kp : 2 * kp + 2, gc0 : gc0 + GW],
                                start=(kp == 0),
                                stop=(kp == DK // 2 - 1),
                                perf_mode=DR,
                            )
                        nc.vector.tensor_add(
                            out=V[:, ms, :, 0:HD],
                            in0=ps[:, 0:GW].rearrange("p (h d) -> p h d", h=HPG),
                            in1=bv_v[:, HPG * g : HPG * (g + 1), :],
                        )

                    # -- attention per head --
                    for hh in range(HPG):
                        m_h = hh // 2
                        p0 = (hh % 2) * 64
                        expS = expp.tile([128, S2 // 128, SQ], f8, tag="expS")
                        for sk in range(S2 // 128):
                            ps = psum.tile([128, 1024], f32, tag="big", bufs=2)
                            for sq in range(2):
                                nc.tensor.matmul(
                                    ps[:, sq * 512 : (sq + 1) * 512],
                                    KT[p0 : p0 + 64, m_h, sk * 128 : (sk + 1) * 128],
                                    QT[p0 : p0 + 64, m_h, sq * 512 : (sq + 1) * 512],
                                    start=True,
                                    stop=True,
                                )
                            nc.scalar.activation(
                                out=expS[:, sk, :],
                                in_=ps,
                                func=AF.Exp,
                                scale=SCALE_EXP,
                                bias=expb_t,
                            )
                        pv = psum.tile([HD + 1, SQ], f32, tag="pv", bufs=2)
                        for sk in range(S2 // 128):
                            for sq in range(2):
                                nc.tensor.matmul(
                                    pv[:, sq * 512 : (sq + 1) * 512],
                                    V[:, sk, hh, 0 : HD + 1],
                                    expS[:, sk, sq * 512 : (sq + 1) * 512],
                                    start=(sk == 0),
                                    stop=(sk == S2 // 128 - 1),
                                )
                        den = nrm.tile([1, SQ], f32, tag="den")
                        nc.vector.tensor_copy(out=den, in_=pv[HD : HD + 1, :])
                        den_bc = nrm.tile([64, SQ], f32, tag="den_bc")
                        nc.gpsimd.partition_broadcast(den_bc, den)
                        nc.vector.reciprocal_approx_fast(out=den_bc, in_=den_bc)
                        h_abs = g * HPG + hh
                        kd = h_abs // 2
                        if h_abs % 2 == 0:
                            nc.vector.tensor_mul(
                                out=xts[0:64, kd, :], in0=pv[0:HD, :], in1=den_bc
                            )
                        else:
                            xt = nrm.tile([64, SQ], f8, tag="xt")
                            nc.vector.tensor_mul(out=xt, in0=pv[0:HD, :], in1=den_bc)
                            nc.sync.dma_start(out=xts[64:128, kd, :], in_=xt)

            # free QKV weights + srcT before the out-projection
            attn_outer.close()

            # ============ out-projection + LN1 ============
            if True:
                with contextlib.ExitStack() as octx:
                    psum = octx.enter_context(
                        tc.tile_pool(name="psC", bufs=1, space="PSUM")
                    )
                    opool = octx.enter_context(tc.tile_pool(name="oproj", bufs=1))
                    otmp = octx.enter_context(tc.tile_pool(name="otmp", bufs=2))

                    g1_bc = opool.tile([128, D], f32)
                    nc.gpsimd.dma_start(out=g1_bc, in_=bc_ap(g1, D))
                    b1_bc = opool.tile([128, D], f32)
                    nc.gpsimd.dma_start(out=b1_bc, in_=bc_ap(b1, D))

                    r_sb = opool.tile([128, SQ // 128, D], f32)
                    accs = {}

                    def pass_a(m):
                        ps = psum.tile([128, 1024], f32, tag="big", bufs=2, name="ops")
                        for n in range(2):
                            for kp in range(DK // 2):
                                nc.tensor.matmul(
                                    ps[:, n * 512 : (n + 1) * 512],
                                    xts[:, 2 * kp : 2 * kp + 2, m * 128 : (m + 1) * 128],
                                    wo_f[:, 2 * kp : 2 * kp + 2, n * 512 : (n + 1) * 512],
                                    start=(kp == 0),
                                    stop=(kp == DK // 2 - 1),
                                    perf_mode=DR,
                                )
                        acc1 = otmp.tile([128, 1], f32, tag="acc1", bufs=4)
                        nc.vector.scalar_tensor_tensor(
                            out=r_sb[:, m, :],
                            in0=ps,
                            scalar=1.0 / (WS * WS),
                            in1=src_q_sb[:, m, :],
                            op0=ALU.mult,
                            op1=ALU.add,
                            accum_out=acc1,
                        )
                        sq_scr = otmp.tile([128, D], f32, tag="sq_scr", bufs=2)
                        acc2 = otmp.tile([128, 1], f32, tag="acc2", bufs=4)
                        nc.scalar.activation(
                            out=sq_scr,
                            in_=r_sb[:, m, :],
                            func=AF.Square,
                            accum_out=acc2,
                        )
                        accs[m] = (acc1, acc2)

                    def pass_b(m):
                        rrow = r_sb[:, m, :]
                        s1 = src1_sb[:, m, :]
                        acc1, acc2 = accs.pop(m)
                        ln_apply(rrow, acc1, acc2, g1_bc, b1_bc, s1, otmp)
                        for kk in range(2):
                            ps = psum.tile(
                                [128, 512], bf16, tag="tpsb", bufs=2, name="tps"
                            )
                            for j in range(4):
                                k = kk * 4 + j
                                nc.tensor.transpose(
                                    ps[:, j * 128 : (j + 1) * 128],
                                    s1[:, k * 128 : (k + 1) * 128],
                                    identity,
                                )
                            nc.scalar.copy(
                                out=src1T[
                                    :, kk * 4 : (kk + 1) * 4, m * 128 : (m + 1) * 128
                                ],
                                in_=ps.rearrange("p (a b) -> p a b", a=4),
                            )

                    for m in range(SQ // 128 + 2):
                        if m < SQ // 128:
                            pass_a(m)
                        if m >= 2:
                            pass_b(m - 2)

                # free src_q/xts/wo before the FFN
                mid.close()

                # ============ FFN ============
                with contextlib.ExitStack() as fctx:
                    psum = fctx.enter_context(
                        tc.tile_pool(name="psD", bufs=1, space="PSUM")
                    )
                    hpool = fctx.enter_context(tc.tile_pool(name="hpool", bufs=1))
                    hT = hpool.tile([128, PFK, SQ], bf16)     # 8MB
                    w2p = fctx.enter_context(tc.tile_pool(name="w2p", bufs=1))
                    w2bf = w2p.tile([128, PFK, D], bf16)      # 8MB
                    fcts = fctx.enter_context(tc.tile_pool(name="fcts", bufs=1))
                    ftmp = fctx.enter_context(tc.tile_pool(name="ftmp", bufs=2))

                    bf2_bc = fcts.tile([128, D], f32)
                    nc.gpsimd.dma_start(out=bf2_bc, in_=bc_ap(bf2, D))
                    g2_bc = fcts.tile([128, D], f32)
                    nc.gpsimd.dma_start(out=g2_bc, in_=bc_ap(g2, D))
                    b2_bc = fcts.tile([128, D], f32)
                    nc.gpsimd.dma_start(out=b2_bc, in_=bc_ap(b2, D))

                    for kb in range(8):
                        nc.gpsimd.dma_start(
                            out=w2bf[:, kb * 4 : (kb + 1) * 4, :],
                            in_=W2.rearrange("(a p) n -> p a n", p=128)[
                                :, kb * 4 : (kb + 1) * 4, :
                            ],
                        )

                    # FFN1: hT[pf, q] = relu(W1^T src1T + bf1); W1 fetched once
                    for mp in range(PFK):
                        w1_s = ftmp.tile([128, DK, 128], bf16, tag="w1_s", bufs=4)
                        nc.sync.dma_start(
                            out=w1_s,
                            in_=W1.rearrange("(a p) n -> p a n", p=128)[
                                :, :, mp * 128 : (mp + 1) * 128
                            ],
                        )
                        for sqh in range(2):
                            ps = psum.tile([128, 512], f32, tag="big", bufs=2)
                            for k in range(DK):
                                nc.tensor.matmul(
                                    ps,
                                    w1_s[:, k, :],
                                    src1T[:, k, sqh * 512 : (sqh + 1) * 512],
                                    start=(k == 0),
                                    stop=(k == DK - 1),
                                )
                            nc.scalar.activation(
                                out=hT[:, mp, sqh * 512 : (sqh + 1) * 512],
                                in_=ps,
                                func=AF.Relu,
                                bias=bf1_col[:, mp : mp + 1],
                                scale=1.0,
                            )

                    # FFN2 per m row + residual + LN2
                    for m in range(SQ // 128):
                        ps = psum.tile([128, 1024], f32, tag="big", bufs=2)
                        for k in range(PFK):
                            for n in range(2):
                                nc.tensor.matmul(
                                    ps[:, n * 512 : (n + 1) * 512],
                                    hT[:, k, m * 128 : (m + 1) * 128],
                                    w2bf[:, k, n * 512 : (n + 1) * 512],
                                    start=(k == 0),
                                    stop=(k == PFK - 1),
                                )
                        rr = ftmp.tile([128, D], f32, tag="rr", bufs=2)
                        nc.vector.tensor_add(out=rr, in0=ps, in1=src1_sb[:, m, :])
                        acc1 = ftmp.tile([128, 1], f32, tag="acc1", bufs=3)
                        nc.vector.scalar_tensor_tensor(
                            out=rr,
                            in0=rr,
                            scalar=0.0,
                            in1=bf2_bc,
                            op0=ALU.add,
                            op1=ALU.add,
                            accum_out=acc1,
                        )
                        sq_scr = ftmp.tile([128, D], f32, tag="sq_scr", bufs=2)
                        acc2 = ftmp.tile([128, 1], f32, tag="acc2", bufs=3)
                        nc.scalar.activation(
                            out=sq_scr, in_=rr, func=AF.Square, accum_out=acc2
                        )
                        ln_apply(rr, acc1, acc2, g2_bc, b2_bc, rr, ftmp)
                        nc.sync.dma_start(out=out[m * 128 : (m + 1) * 128, :], in_=rr)

    nc.compile()
    return nc


def _prep_in_maps(ins):
    import ml_dtypes

    bf = ml_dtypes.bfloat16
    f8 = ml_dtypes.float8_e4m3
    src = ins["src"]
    weights = {}
    for n in ["Wq", "Wk", "Wv", "Wo"]:
        weights[n] = np.ascontiguousarray(ins[n] * WS).astype(f8)
    for n in ["W1", "W2"]:
        weights[n] = np.ascontiguousarray(ins[n]).astype(bf)
    for n in ["bq", "bk", "bv"]:
        weights[n] = np.ascontiguousarray(ins[n] * WS)
    for n in ["bo", "bf1", "bf2", "g1", "b1", "g2", "b2"]:
        weights[n] = np.ascontiguousarray(ins[n])

    in_maps = []
    for c in range(N_CORES):
        b, h = divmod(c, 2)
        m = dict(weights)
        m["src_q"] = np.ascontiguousarray(src[b, h * SQ : (h + 1) * SQ]).astype(bf)
        m["src_o"] = np.ascontiguousarray(src[b, (1 - h) * SQ : (2 - h) * SQ]).astype(bf)
        in_maps.append(m)
    return in_maps


def kernel(**inputs):
    from concourse.bass_utils import run_bass_kernel_spmd

    if "nc" not in _CACHE:
        _CACHE["nc"] = _build()
    nc = _CACHE["nc"]

    ins = {k: np.asarray(v, dtype=np.float32) for k, v in inputs.items()}
    in_maps = _prep_in_maps(ins)

    res = run_bass_kernel_spmd(nc, in_maps, list(range(N_CORES)))

    out = np.empty((4, S2, D), dtype=np.float32)
    for c in range(N_CORES):
        b, h = divmod(c, 2)
        out[b, h * SQ : (h + 1) * SQ] = res.results[c]["out"]
    return out


# revision 53
# speedup vs baseline: 1.2049x; 1.2049x over previous
"""Trainium2 Bass kernel for a transformer encoder layer.

B=4, S=2048, D=1024, H=16 heads (HD=64), PF=4096, fp32 I/O.

Sharding: 8 cores, core c handles batch c//2, query seq-half c%2 (1024
tokens). Each core computes K/V over its batch's full 2048-token sequence
(duplicated within the pair; ~12% extra flops) so no collectives are needed.

Precision plan:
- Attention path in fp8e4 with DoubleRow matmuls (2 contraction subtiles
  per instruction, 2x PE throughput): QKV projections, attn@V, out-proj.
  Wq/Wk/Wv/Wo are scaled x32 host-side so their values use the fp8 range;
  the compensating 1/1024 factors are folded into the exp scale (scores)
  and the out-proj eviction. Scores (64-deep contraction) stay bf16 -
  DoubleRow cannot speed a 64-row contraction.
- exp computed as exp(s/8 - 2) so expS fits fp8e4 (max 240); the -2 shift
  cancels in the softmax normalization (ones-column denominator trick).
- FFN and residual/LN stay bf16/fp32 for accuracy.
- src streamed in as bf16; transposes run in bf16 (1 cyc/row vs 2 for f32).
- x^T and src1 stay SBUF-resident (no DRAM round trips).
"""

import numpy as np

D = 1024
S2 = 2048
SQ = 1024
PF = 4096
H = 16
HD = 64
DK = D // 128
PFK = PF // 128
NG = 4                 # head groups
HPG = H // NG
GW = HPG * HD          # 256 dims per group
GM = GW // 128
WS = 32.0              # host-side weight scale for fp8 Wq/Wk/Wv/Wo
SCALE_EXP = 1.0 / (8.0 * WS * WS)   # undo q,k scales and apply 1/sqrt(HD)
EXP_BIAS = -2.0
EPS = 1e-5
N_CORES = 8

_CACHE = {}


def _build():
    import concourse.bass as bass
    import concourse.mybir as mybir
    import concourse.tile as tile
    from concourse import bacc
    from concourse.masks import make_identity

    f32 = mybir.dt.float32
    bf16 = mybir.dt.bfloat16
    f8 = mybir.dt.float8e4
    AF = mybir.ActivationFunctionType
    ALU = mybir.AluOpType
    DR = mybir.MatmulPerfMode.DoubleRow

    nc = bacc.Bacc("TRN2", target_bir_lowering=False, debug=False, num_devices=N_CORES)

    def din(name, shape, dt=f32):
        return nc.dram_tensor(name, shape, dt, kind="ExternalInput")

    src_q = din("src_q", [SQ, D], bf16)
    src_o = din("src_o", [SQ, D], bf16)
    Wq = din("Wq", [D, D], f8)
    Wk = din("Wk", [D, D], f8)
    Wv = din("Wv", [D, D], f8)
    Wo = din("Wo", [D, D], f8)
    W1 = din("W1", [D, PF], bf16)
    W2 = din("W2", [PF, D], bf16)
    bq = din("bq", [D])
    bk = din("bk", [D])
    bv = din("bv", [D])
    bo = din("bo", [D])
    bf1 = din("bf1", [PF])
    bf2 = din("bf2", [D])
    g1 = din("g1", [D])
    b1 = din("b1", [D])
    g2 = din("g2", [D])
    b2 = din("b2", [D])
    out = nc.dram_tensor("out", [SQ, D], f32, kind="ExternalOutput")

    def bc_ap(vec, n):
        return bass.AP(tensor=vec, offset=0, ap=[[0, 128], [1, n]])

    def col_ap(vec, m):
        return bass.AP(tensor=vec, offset=0, ap=[[1, 128], [128, m]])

    with tile.TileContext(nc) as tc:
        import contextlib

        with contextlib.ExitStack() as ctx:
            consts = ctx.enter_context(tc.tile_pool(name="consts", bufs=1))

            identity = consts.tile([128, 128], bf16)
            make_identity(nc, identity)

            bq_col = consts.tile([128, DK], f32)
            nc.sync.dma_start(out=bq_col, in_=col_ap(bq, DK))
            bk_col = consts.tile([128, DK], f32)
            nc.sync.dma_start(out=bk_col, in_=col_ap(bk, DK))
            bf1_col = consts.tile([128, PFK], f32)
            nc.sync.dma_start(out=bf1_col, in_=col_ap(bf1, PFK))

            eps_t = consts.tile([128, 1], f32)
            nc.vector.memset(eps_t, EPS)
            expb_t = consts.tile([128, 1], f32)
            nc.vector.memset(expb_t, EXP_BIAS)

            # src1 / src1^T span out-proj -> FFN; allocated up front
            src1p = ctx.enter_context(tc.tile_pool(name="src1p", bufs=1))
            src1_sb = src1p.tile([128, SQ // 128, D], bf16)  # 2MB
            src1T = src1p.tile([128, DK, SQ], bf16)          # 2MB

            mid = ctx.enter_context(contextlib.ExitStack())
            # activations that span phase0 -> out-proj (freed before FFN):
            # src_q (+bo) residual (bf16), x^T (fp8), Wo (fp8)
            actp = mid.enter_context(tc.tile_pool(name="actp", bufs=1))
            src_q_sb = actp.tile([128, SQ // 128, D], bf16)  # 2MB
            xts = actp.tile([128, DK, SQ], f8)        # 1MB
            wo_f = actp.tile([128, DK, D], f8)
            nc.gpsimd.dma_start(out=wo_f, in_=Wo.rearrange("(a p) n -> p a n", p=128))

            def ln_apply(r_row, acc1, acc2, g_bc, b_bc, out_tile, tmp_pool):
                """LayerNorm given acc1=sum(r) and acc2=sum(r^2) per partition.

                The moment sums come for free from accum_out on the passes
                that produce r, so this needs only two full-width DVE passes.
                The tiny [128,1] moment arithmetic runs on gpsimd (idle).
                """
                mu = tmp_pool.tile([128, 1], f32, tag="ln_mu")
                nc.vector.tensor_scalar_mul(out=mu, in0=acc1, scalar1=1.0 / D)
                musq = tmp_pool.tile([128, 1], f32, tag="ln_musq")
                nc.vector.tensor_mul(out=musq, in0=mu, in1=mu)
                var = tmp_pool.tile([128, 1], f32, tag="ln_var")
                nc.vector.scalar_tensor_tensor(
                    out=var,
                    in0=acc2,
                    scalar=1.0 / D,
                    in1=musq,
                    op0=ALU.mult,
                    op1=ALU.subtract,
                )
                rstd = tmp_pool.tile([128, 1], f32, tag="ln_rstd")
                nc.scalar.activation(
                    out=rstd, in_=var, func=AF.Sqrt, bias=eps_t, scale=1.0
                )
                nc.vector.reciprocal_approx_fast(out=rstd, in_=rstd)
                nc.vector.scalar_tensor_tensor(
                    out=out_tile,
                    in0=r_row,
                    scalar=mu,
                    in1=g_bc,
                    op0=ALU.subtract,
                    op1=ALU.mult,
                )
                nc.vector.scalar_tensor_tensor(
                    out=out_tile,
                    in0=out_tile,
                    scalar=rstd,
                    in1=b_bc,
                    op0=ALU.mult,
                    op1=ALU.add,
                )

            # attention-scope tensors: QKV weights (fp8), src^T (fp8), bo
            attn_outer = mid.enter_context(contextlib.ExitStack())
            wqkvp = attn_outer.enter_context(tc.tile_pool(name="wqkvp", bufs=1))
            wk_f = wqkvp.tile([128, DK, D], f8)
            nc.sync.dma_start(out=wk_f, in_=Wk.rearrange("(a p) n -> p a n", p=128))
            wq_f = wqkvp.tile([128, DK, D], f8)
            nc.sync.dma_start(out=wq_f, in_=Wq.rearrange("(a p) n -> p a n", p=128))
            wv_f = wqkvp.tile([128, DK, D], f8)
            nc.sync.dma_start(out=wv_f, in_=Wv.rearrange("(a p) n -> p a n", p=128))
            srcT = wqkvp.tile([128, DK, S2], f8)      # 2MB
            bo_bc = wqkvp.tile([128, D], f32)
            nc.gpsimd.dma_start(out=bo_bc, in_=bc_ap(bo, D))

            # ============ Phase 0: transpose src (bf16 in, fp8 out) ============
            with tc.tile_pool(name="psA", bufs=1, space="PSUM") as psA:
                # PE warm-up: ramp the clock while the first src DMAs land.
                for w in range(22):
                    wps = psA.tile([128, 512], bf16, tag="tpsb", bufs=4)
                    for j in range(4):
                        nc.tensor.transpose(
                            wps[:, j * 128 : (j + 1) * 128], identity, identity
                        )

                with tc.tile_pool(name="ph0", bufs=2) as ph0:
                    for blk in range(4):
                        sts = []
                        for j in range(4):
                            row0 = (blk % 2) * 512 + j * 128
                            if blk < 2:
                                st = src_q_sb[:, blk * 4 + j, :]
                                nc.sync.dma_start(
                                    out=st, in_=src_q[row0 : row0 + 128, :]
                                )
                            else:
                                st = ph0.tile([128, D], bf16, tag="src_ld", bufs=8)
                                nc.sync.dma_start(
                                    out=st, in_=src_o[row0 : row0 + 128, :]
                                )
                            sts.append(st)
                        base = blk * 512
                        for k in range(DK):
                            ps = psA.tile([128, 512], bf16, tag="tpsb", bufs=4)
                            for j in range(4):
                                nc.tensor.transpose(
                                    ps[:, j * 128 : (j + 1) * 128],
                                    sts[j][:, k * 128 : (k + 1) * 128],
                                    identity,
                                )
                            nc.vector.tensor_copy(
                                out=srcT[:, k, base : base + 512], in_=ps
                            )
                # fold bo into the resident src_q copy (after its transposes)
                for j in range(SQ // 128):
                    nc.vector.tensor_add(
                        out=src_q_sb[:, j, :], in0=src_q_sb[:, j, :], in1=bo_bc
                    )

            # ============ attention ============
            with contextlib.ExitStack() as attn_ctx:
                psum = attn_ctx.enter_context(
                    tc.tile_pool(name="psB", bufs=1, space="PSUM")
                )
                acts = attn_ctx.enter_context(tc.tile_pool(name="acts", bufs=1))
                bv_bc = acts.tile([128, D], f32)
                nc.gpsimd.dma_start(out=bv_bc, in_=bc_ap(bv, D))

                grp = attn_ctx.enter_context(tc.tile_pool(name="grp", bufs=2))
                expp = attn_ctx.enter_context(tc.tile_pool(name="expp", bufs=2))
                nrm = attn_ctx.enter_context(tc.tile_pool(name="nrm", bufs=2))

                for g in range(NG):
                    gc0 = g * GW

                    # -- KT_g [GW, S2] fp8 (values x32) --
                    KT = grp.tile([128, GM, S2], f8, tag="KT")
                    for m in range(GM):
                        mc = gc0 + m * 128
                        for nn in range(S2 // 1024):
                            ps = psum.tile([128, 1024], f32, tag="big", bufs=2)
                            for hf in range(2):
                                c0 = nn * 1024 + hf * 512
                                for kp in range(DK // 2):
                                    nc.tensor.matmul(
                                        ps[:, hf * 512 : (hf + 1) * 512],
                                        wk_f[:, 2 * kp : 2 * kp + 2, mc : mc + 128],
                                        srcT[:, 2 * kp : 2 * kp + 2, c0 : c0 + 512],
                                        start=(kp == 0),
                                        stop=(kp == DK // 2 - 1),
                                        perf_mode=DR,
                                    )
                            nc.vector.tensor_scalar_add(
                                out=KT[:, m, nn * 1024 : (nn + 1) * 1024],
                                in0=ps,
                                scalar1=bk_col[:, mc // 128 : mc // 128 + 1],
                            )

                    # -- QT_g [GW, SQ] fp8 (values x32) --
                    QT = grp.tile([128, GM, SQ], f8, tag="QT")
                    for m in range(GM):
                        mc = gc0 + m * 128
                        ps = psum.tile([128, 1024], f32, tag="big", bufs=2)
                        for hf in range(2):
                            for kp in range(DK // 2):
                                nc.tensor.matmul(
                                    ps[:, hf * 512 : (hf + 1) * 512],
                                    wq_f[:, 2 * kp : 2 * kp + 2, mc : mc + 128],
                                    srcT[:, 2 * kp : 2 * kp + 2, hf * 512 : (hf + 1) * 512],
                                    start=(kp == 0),
                                    stop=(kp == DK // 2 - 1),
                                    perf_mode=DR,
                                )
                        nc.vector.tensor_scalar_add(
                            out=QT[:, m, :],
                            in0=ps,
                            scalar1=bq_col[:, mc // 128 : mc // 128 + 1],
                        )

                    # -- V_g fp8 [S2, HPG, 80], each head [v|1|pad] --
                    # (stride 80 keeps the DoubleRow pair step 16B-aligned)
                    VP = 80
                    V = grp.tile([128, S2 // 128, HPG, VP], f8, tag="V")
                    nc.vector.memset(V[:, :, :, HD : HD + 1], 1.0)
                    bv_v = bv_bc.rearrange("p (h d) -> p h d", h=H)
                    for ms in range(S2 // 128):
                        ps = psum.tile([128, 1024], f32, tag="big", bufs=2)
                        for kp in range(DK // 2):
                            nc.tensor.matmul(
                                ps[:, 0:GW],
                                srcT[:, 2 * kp : 2 * kp + 2, ms * 128 : (ms + 1) * 128],
                                wv_f[:, 2 * kp : 2 * kp + 2, gc0 : gc0 + GW],
                                start=(kp == 0),
                                stop=(kp == DK // 2 - 1),
                                perf_mode=DR,
                            )
                        nc.vector.tensor_add(
                            out=V[:, ms, :, 0:HD],
                            in0=ps[:, 0:GW].rearrange("p (h d) -> p h d", h=HPG),
                            in1=bv_v[:, HPG * g : HPG * (g + 1), :],
                        )

                    # -- attention per head --
                    for hh in range(HPG):
                        m_h = hh // 2
                        p0 = (hh % 2) * 64
                        expS = expp.tile([128, S2 // 128, SQ], f8, tag="expS")
                        for sk in range(S2 // 128):
                            ps = psum.tile([128, 1024], f32, tag="big", bufs=2)
                            for sq in range(2):
                                nc.tensor.matmul(
                                    ps[:, sq * 512 : (sq + 1) * 512],
                                    KT[p0 : p0 + 64, m_h, sk * 128 : (sk + 1) * 128],
                                    QT[p0 : p0 + 64, m_h, sq * 512 : (sq + 1) * 512],
                                    start=True,
                                    stop=True,
                                )
                            nc.scalar.activation(
                                out=expS[:, sk, :],
                                in_=ps,
                                func=AF.Exp,
                                scale=SCALE_EXP,
                                bias=expb_t,
                            )
                        pv = psum.tile([HD + 1, SQ], f32, tag="pv", bufs=2)
                        for sk in range(S2 // 128):
                            for sq in range(2):
                                nc.tensor.matmul(
                                    pv[:, sq * 512 : (sq + 1) * 512],
                                    V[:, sk, hh, 0 : HD + 1],
                                    expS[:, sk, sq * 512 : (sq + 1) * 512],
                                    start=(sk == 0),
                                    stop=(sk == S2 // 128 - 1),
                                )
                        den = nrm.tile([1, SQ], f32, tag="den")
                        nc.vector.tensor_copy(out=den, in_=pv[HD : HD + 1, :])
                        den_bc = nrm.tile([64, SQ], f32, tag="den_bc")
                        nc.gpsimd.partition_broadcast(den_bc, den)
                        nc.vector.reciprocal_approx_fast(out=den_bc, in_=den_bc)
                        h_abs = g * HPG + hh
                        kd = h_abs // 2
                        if h_abs % 2 == 0:
                            nc.vector.tensor_mul(
                                out=xts[0:64, kd, :], in0=pv[0:HD, :], in1=den_bc
                            )
                        else:
                            xt = nrm.tile([64, SQ], f8, tag="xt")
                            nc.vector.tensor_mul(out=xt, in0=pv[0:HD, :], in1=den_bc)
                            nc.sync.dma_start(out=xts[64:128, kd, :], in_=xt)

            # free QKV weights + srcT before the out-projection
            attn_outer.close()

            # ============ out-projection + LN1 ============
            if True:
                with contextlib.ExitStack() as octx:
                    psum = octx.enter_context(
                        tc.tile_pool(name="psC", bufs=1, space="PSUM")
                    )
                    opool = octx.enter_context(tc.tile_pool(name="oproj", bufs=1))
                    otmp = octx.enter_context(tc.tile_pool(name="otmp", bufs=2))

                    g1_bc = opool.tile([128, D], f32)
                    nc.gpsimd.dma_start(out=g1_bc, in_=bc_ap(g1, D))
                    b1_bc = opool.tile([128, D], f32)
                    nc.gpsimd.dma_start(out=b1_bc, in_=bc_ap(b1, D))

                    r_sb = opool.tile([128, SQ // 128, D], f32)
                    accs = {}

                    def pass_a(m):
                        ps = psum.tile([128, 1024], f32, tag="big", bufs=2, name="ops")
                        for n in range(2):
                            for kp in range(DK // 2):
                                nc.tensor.matmul(
                                    ps[:, n * 512 : (n + 1) * 512],
                                    xts[:, 2 * kp : 2 * kp + 2, m * 128 : (m + 1) * 128],
                                    wo_f[:, 2 * kp : 2 * kp + 2, n * 512 : (n + 1) * 512],
                                    start=(kp == 0),
                                    stop=(kp == DK // 2 - 1),
                                    perf_mode=DR,
                                )
                        acc1 = otmp.tile([128, 1], f32, tag="acc1", bufs=4)
                        nc.vector.scalar_tensor_tensor(
                            out=r_sb[:, m, :],
                            in0=ps,
                            scalar=1.0 / (WS * WS),
                            in1=src_q_sb[:, m, :],
                            op0=ALU.mult,
                            op1=ALU.add,
                            accum_out=acc1,
                        )
                        sq_scr = otmp.tile([128, D], f32, tag="sq_scr", bufs=2)
                        acc2 = otmp.tile([128, 1], f32, tag="acc2", bufs=4)
                        nc.scalar.activation(
                            out=sq_scr,
                            in_=r_sb[:, m, :],
                            func=AF.Square,
                            accum_out=acc2,
                        )
                        accs[m] = (acc1, acc2)

                    def pass_b(m):
                        rrow = r_sb[:, m, :]
                        s1 = src1_sb[:, m, :]
                        acc1, acc2 = accs.pop(m)
                        ln_apply(rrow, acc1, acc2, g1_bc, b1_bc, s1, otmp)
                        for kk in range(2):
                            ps = psum.tile(
                                [128, 512], bf16, tag="tpsb", bufs=2, name="tps"
                            )
                            for j in range(4):
                                k = kk * 4 + j
                                nc.tensor.transpose(
                                    ps[:, j * 128 : (j + 1) * 128],
                                    s1[:, k * 128 : (k + 1) * 128],
                                    identity,
                                )
                            nc.scalar.copy(
                                out=src1T[
                                    :, kk * 4 : (kk + 1) * 4, m * 128 : (m + 1) * 128
                                ],
                                in_=ps.rearrange("p (a b) -> p a b", a=4),
                            )

                    for m in range(SQ // 128 + 2):
                        if m < SQ // 128:
                            pass_a(m)
                        if m >= 2:
                            pass_b(m - 2)

                # free src_q/xts/wo before the FFN
                mid.close()

                # ============ FFN ============
                with contextlib.ExitStack() as fctx:
                    psum = fctx.enter_context(
                        tc.tile_pool(name="psD", bufs=1, space="PSUM")
                    )
                    hpool = fctx.enter_context(tc.tile_pool(name="hpool", bufs=1))
                    hT = hpool.tile([128, PFK, SQ], bf16)     # 8MB
                    w2p = fctx.enter_context(tc.tile_pool(name="w2p", bufs=1))
                    w2bf = w2p.tile([128, PFK, D], bf16)      # 8MB
                    fcts = fctx.enter_context(tc.tile_pool(name="fcts", bufs=1))
                    ftmp = fctx.enter_context(tc.tile_pool(name="ftmp", bufs=2))

                    bf2_bc = fcts.tile([128, D], f32)
                    nc.gpsimd.dma_start(out=bf2_bc, in_=bc_ap(bf2, D))
                    g2_bc = fcts.tile([128, D], f32)
                    nc.gpsimd.dma_start(out=g2_bc, in_=bc_ap(g2, D))
                    b2_bc = fcts.tile([128, D], f32)
                    nc.gpsimd.dma_start(out=b2_bc, in_=bc_ap(b2, D))

                    for kb in range(8):
                        nc.gpsimd.dma_start(
                            out=w2bf[:, kb * 4 : (kb + 1) * 4, :],
                            in_=W2.rearrange("(a p) n -> p a n", p=128)[
                                :, kb * 4 : (kb + 1) * 4, :
                            ],
                        )

                    # FFN1: hT[pf, q] = relu(W1^T src1T + bf1); W1 fetched once
                    for mp in range(PFK):
                        w1_s = ftmp.tile([128, DK, 128], bf16, tag="w1_s", bufs=4)
                        nc.sync.dma_start(
                            out=w1_s,
                            in_=W1.rearrange("(a p) n -> p a n", p=128)[
                                :, :, mp * 128 : (mp + 1) * 128
                            ],
                        )
                        for sqh in range(2):
                            ps = psum.tile([128, 512], f32, tag="big", bufs=2)
                            for k in range(DK):
                                nc.tensor.matmul(
                                    ps,
                                    w1_s[:, k, :],
                                    src1T[:, k, sqh * 512 : (sqh + 1) * 512],
                                    start=(k == 0),
                                    stop=(k == DK - 1),
                                )
                            nc.scalar.activation(
                                out=hT[:, mp, sqh * 512 : (sqh + 1) * 512],
                                in_=ps,
                                func=AF.Relu,
                                bias=bf1_col[:, mp : mp + 1],
                                scale=1.0,
                            )

                    # FFN2 per m row + residual + LN2
                    for m in range(SQ // 128):
                        ps = psum.tile([128, 1024], f32, tag="big", bufs=2)
                        for k in range(PFK):
                            for n in range(2):
                                nc.tensor.matmul(
                                    ps[:, n * 512 : (n + 1) * 512],
                                    hT[:, k, m * 128 : (m + 1) * 128],
                                    w2bf[:, k, n * 512 : (n + 1) * 512],
                                    start=(k == 0),
                                    stop=(k == PFK - 1),
                                )
                        rr = ftmp.tile([128, D], f32, tag="rr", bufs=2)
                        nc.vector.tensor_add(out=rr, in0=ps, in1=src1_sb[:, m, :])
                        acc1 = ftmp.tile([128, 1], f32, tag="acc1", bufs=3)
                        nc.vector.scalar_tensor_tensor(
                            out=rr,
                            in0=rr,
                            scalar=0.0,
                            in1=bf2_bc,
                            op0=ALU.add,
                            op1=ALU.add,
                            accum_out=acc1,
                        )
                        sq_scr = ftmp.tile([128, D], f32, tag="sq_scr", bufs=2)
                        acc2 = ftmp.tile([128, 1], f32, tag="acc2", bufs=3)
                        nc.scalar.activation(
                            out=sq_scr, in_=rr, func=AF.Square, accum_out=acc2
                        )
                        ln_apply(rr, acc1, acc2, g2_bc, b2_bc, rr, ftmp)
                        nc.sync.dma_start(out=out[m * 128 : (m + 1) * 128, :], in_=rr)

    nc.compile()
    return nc


def _prep_in_maps(ins):
    import ml_dtypes

    bf = ml_dtypes.bfloat16
    f8 = ml_dtypes.float8_e4m3
    src = ins["src"]
    weights = {}
    for n in ["Wq", "Wk", "Wv", "Wo"]:
        weights[n] = np.ascontiguousarray(ins[n] * WS).astype(f8)
    for n in ["W1", "W2"]:
        weights[n] = np.ascontiguousarray(ins[n]).astype(bf)
    for n in ["bq", "bk", "bv"]:
        weights[n] = np.ascontiguousarray(ins[n] * WS)
    for n in ["bo", "bf1", "bf2", "g1", "b1", "g2", "b2"]:
        weights[n] = np.ascontiguousarray(ins[n])

    in_maps = []
    for c in range(N_CORES):
        b, h = divmod(c, 2)
        m = dict(weights)
        m["src_q"] = np.ascontiguousarray(src[b, h * SQ : (h + 1) * SQ]).astype(bf)
        m["src_o"] = np.ascontiguousarray(src[b, (1 - h) * SQ : (2 - h) * SQ]).astype(bf)
        in_maps.append(m)
    return in_maps


def kernel(**inputs):
    from concourse.bass_utils import run_bass_kernel_spmd

    if "nc" not in _CACHE:
        _CACHE["nc"] = _build()
    nc = _CACHE["nc"]

    ins = {k: np.asarray(v, dtype=np.float32) for k, v in inputs.items()}
    in_maps = _prep_in_maps(ins)

    res = run_bass_kernel_spmd(nc, in_maps, list(range(N_CORES)))

    out = np.empty((4, S2, D), dtype=np.float32)
    for c in range(N_CORES):
        b, h = divmod(c, 2)
        out[b, h * SQ : (h + 1) * SQ] = res.results[c]["out"]
    return out
